# revision 14
# baseline (speedup 1.0000x reference)
"""Trainium2 Bass kernel for nn_Attention_10282151707309.

Reference computation:
  - channel LayerNorm over C=128 (biased var, eps=1e-5, affine g/b)
  - qkv = w_qkv @ xn (1x1 conv), 4 heads x 32 dims, q scaled by 1/sqrt(32)
  - full softmax attention over HW=4096 positions per (batch, head)
  - out = w_out @ attn_out + b_out

Sharding: 8 cores = (batch b in 0..3) x (spatial half in 0..1).
Each core runs an IDENTICAL program; per-core inputs differ:
  - x is the batch slice, spatially rolled so the core's own 2048 query
    columns are always program-columns 0:2048 (attention is permutation-
    equivariant over key positions, so the roll is harmless).
  - every core computes LN + k/v over all 4096 positions of its batch
    (2x redundant per batch, cheap) and q only over its own half.
No collectives; each core writes a disjoint slice of the output.

Performance model (from the baseline's perfetto trace):
  - PE streams 512-col bf16 matmuls at 215ns issue-to-issue (2.4GHz, one
    column/cycle) with LDWEIGHTS fully hidden under the previous matmul.
    sim+av = 524288 columns ~ 220us: the PE has slack.
  - The ACT engine is the wall: exp of 33.5M sim elements at 128 lanes
    @1.2GHz with ~350 cycles fixed cost per ACTIVATE. Everything else
    must stay off ACT and the exp ops must be as large as PSUM allows.
Design consequences:
  - One activation-table preload (natural_log_exp_and_others) up front;
    Ln/Exp both live there, so zero ACT_TABLE_LOADs in steady state
    (the baseline lost 52us to 41 of them).
  - Tail softmax 1/denominator on DVE (reciprocal), not ACT ln/exp.
    All PSUM->SBUF copies and bias adds on DVE; x^2/casts on GPSIMD.
  - k-bias is dropped entirely: sim_h[j,i] += q_h[:,i]@bias_k is
    constant over j, and softmax over j is shift-invariant. Exact.
  - LN + projections are EMISSION-INTERLEAVED with i-tile 0's attention:
    LN s-tile t's stats/bc/proj slot between jc groups, so the exp
    stream starts ~6us into the kernel instead of after a 130us LN
    phase. LN keeps a 2-PSUM-bank footprint (one [128,1024] buffer
    cycled stats -> bc -> q|k proj -> v proj).
  - sim uses K=32 lhsT slices (k_sb rows 32h:32h+32) at tile_position
    (32h, 0); av uses M=33 lhsT slices (v_sb per-chunk [32 dims | ones])
    at col positions 0/64 of the pair banks; no zero-padded k_pad/vaug
    tiles or their 27us of gpsimd memsets.
  - exp tiles: [128,1024] for it0 (PSUM: 2 LN banks + 4 duo + 2 pair),
    [128,1536] for it1-3 (6 duo + 2 pair; the it-tail borrows a duo
    buffer for its bc/y matmuls).
PSUM budget (8 banks of 2KB/partition):
  it0: lnps [128,1024] (2) + duo 2x[128,1024] (4) + pairs 2x[128,512] (2)
  it1+: duo 2x[128,1536] (6) + pairs (2)
"""

import numpy as np

HEADS = 4
DIM_HEAD = 32
B, C, H, W = 4, 128, 64, 64
S = H * W              # 4096 spatial positions
HALF = S // 2          # 2048 own query columns per core
TI = 512               # i-tile (query) size
NIT = HALF // TI       # 4 i-tiles
JCHUNK = 128           # j-chunk (key) size
NJC = S // JCHUNK      # 32 j-chunks
EPS = 1e-5
N_CORES = 8
VW = DIM_HEAD + 1      # 33: v dims + softmax-denominator ones column

_PROGRAM = None


def _build_program():
    """Build the (SPMD-identical) Bass program once per process."""
    import concourse.bass as bass  # noqa: F401
    import concourse.mybir as mybir
    import concourse.tile as tile
    from concourse import bacc
    from concourse.bass import ts

    dt = mybir.dt.float32
    dtb = mybir.dt.bfloat16
    F = mybir.ActivationFunctionType
    Op = mybir.AluOpType

    nc = bacc.Bacc(
        "TRN2",
        target_bir_lowering=False,
        debug=False,
        num_devices=N_CORES,
    )

    x_d = nc.dram_tensor("x", [C, S], dt, kind="ExternalInput").ap()
    wq_d = nc.dram_tensor("wq_t", [C, 128], dtb, kind="ExternalInput").ap()
    wk_d = nc.dram_tensor("wk_t", [C, 128], dtb, kind="ExternalInput").ap()
    wv_d = nc.dram_tensor("wv_t", [C, 128], dtb, kind="ExternalInput").ap()
    woa_d = nc.dram_tensor("wo_a", [97, 128], dt, kind="ExternalInput").ap()
    wob_d = nc.dram_tensor("wo_b", [97, 128], dt, kind="ExternalInput").ap()
    bq_d = nc.dram_tensor("bias_q", [128, 1], dt, kind="ExternalInput").ap()
    bo_d = nc.dram_tensor("bias_o", [128, 1], dt, kind="ExternalInput").ap()
    y_d = nc.dram_tensor("y", [C, HALF], dt, kind="ExternalOutput").ap()

    with tile.TileContext(nc) as tc:
        from contextlib import ExitStack

        with ExitStack() as ctx:
            const_pool = ctx.enter_context(tc.tile_pool(name="const", bufs=1))
            big_pool = ctx.enter_context(tc.tile_pool(name="big", bufs=1))

            # One table set (natural_log_exp_and_others, id 6) serves every
            # activation in this kernel (Exp, Ln); preload it once so the
            # table-placement pass never ping-pongs between the exp-only and
            # ln-only tables.
            nc.scalar.add_instruction(
                mybir.InstLoadActFuncSet(
                    name="act_preload", act_func_set_id=6, ins=[], outs=[]
                )
            )

            wq = const_pool.tile([C, 128], dtb, tag="wq")
            wk = const_pool.tile([C, 128], dtb, tag="wk")
            wv = const_pool.tile([C, 128], dtb, tag="wv")
            woa = const_pool.tile([97, 128], dt, tag="woa")
            wob = const_pool.tile([97, 128], dt, tag="wob")
            bq = const_pool.tile([128, 1], dt, tag="bq")
            bo = const_pool.tile([128, 1], dt, tag="bo")
            ones1 = const_pool.tile([1, 128], dt, tag="ones1")
            onesC = const_pool.tile([128, 1], dtb, tag="onesC")
            # bc lhsT: row 0 selects the even-head reciprocal into out rows
            # 0:33, row 32 the odd-head one into rows 64:97 (engine AP
            # partition bases must be 32-aligned, so the two reciprocal rows
            # live at partitions 0 and 32; rows 1:32 are zero against
            # whatever sits in the rec tile there).
            ones2 = const_pool.tile([33, 97], dt, tag="ones2")
            epsc = const_pool.tile([1, 1], dt, tag="epsc")

            x_sb = big_pool.tile([C, S], dt, tag="x")
            xn = big_pool.tile([C, S], dtb, tag="xn")
            q_sb = big_pool.tile([128, HALF], dtb, tag="q")
            k_sb = big_pool.tile([128, S], dtb, tag="k")
            # v_sb: per j-chunk, per head: [32 dims | ones] = 33 cols
            v_sb = big_pool.tile([128, NJC * HEADS * VW], dtb, tag="v")
            # Matmul APs may only start at partition 0/32/64, so head 3's
            # K=32 sim (k rows 96:128) is expressed as a K=64 matmul at base
            # 64 against this padded q copy whose rows 64:96 are zero
            # (killing head 2's k rows exactly).
            qp3 = big_pool.tile([128, HALF], dtb, tag="qp3")
            catA = big_pool.tile([128, TI], dt, tag="catA")
            catB = big_pool.tile([128, TI], dt, tag="catB")
            recA = big_pool.tile([33, TI], dt, tag="recA")
            recB = big_pool.tile([33, TI], dt, tag="recB")

            # input DMAs: x s-tile 0 first (heads the LN pipeline), then the
            # small weights, then the rest of x.
            nc.sync.dma_start(x_sb[:, 0:512], x_d[:, 0:512])
            nc.sync.dma_start(wq[:], wq_d[:])
            nc.sync.dma_start(wk[:], wk_d[:])
            nc.sync.dma_start(wv[:], wv_d[:])
            nc.sync.dma_start(woa[:], woa_d[:])
            nc.sync.dma_start(wob[:], wob_d[:])
            nc.sync.dma_start(bq[:], bq_d[:])
            nc.sync.dma_start(bo[:], bo_d[:])
            for t in range(1, 8):
                nc.sync.dma_start(x_sb[:, ts(t, 512)], x_d[:, ts(t, 512)])

            nc.vector.memset(ones1[:], 1.0)
            nc.vector.memset(onesC[:], 1.0 / C)
            nc.vector.memset(ones2[:], 0.0)
            nc.vector.memset(ones2[0:1, 0:33], 1.0)
            nc.vector.memset(ones2[32:33, 64:97], 1.0)
            nc.vector.memset(recA[:], 0.0)
            nc.vector.memset(recB[:], 0.0)
            nc.vector.memset(epsc[:], EPS)
            # softmax-denominator ones column of v_sb
            vones = v_sb[:].rearrange(
                "p (c h e) -> p c h e", h=HEADS, e=VW
            )[:, :, :, DIM_HEAD : DIM_HEAD + 1]
            nc.vector.memset(vones, 1.0)
            # cat rows 33:64 are never written but are read by the K=97
            # y matmul (against zero rows of wo) -- must not be NaN.
            # (row 32 is rewritten by every tail; zeroing from 32 keeps the
            # memset partition base 32-aligned.)
            nc.vector.memset(catA[32:64, :], 0.0)
            nc.vector.memset(catB[32:64, :], 0.0)
            nc.vector.memset(qp3[64:96, :], 0.0)

            pair_pool = ctx.enter_context(
                tc.tile_pool(name="pair_ps", bufs=2, space="PSUM")
            )
            expo_pool = ctx.enter_context(tc.tile_pool(name="expo", bufs=3))
            rec_pool = ctx.enter_context(tc.tile_pool(name="rec", bufs=2))
            ysb_pool = ctx.enter_context(tc.tile_pool(name="ysb", bufs=2))
            sm_pool = ctx.enter_context(tc.tile_pool(name="lnsm", bufs=2))
            gx_pool = ctx.enter_context(tc.tile_pool(name="lngx", bufs=2))

            # ---------------- LayerNorm + projections ----------------
            # Per s-tile (512 positions), using one [128,1024] PSUM buffer
            # cycled through 4 generations: stats -> bc -> (qp|kp) -> vp.
            # Emission is split into three slots (A/B/C) that interleave
            # with i-tile 0's attention groups.
            ln_state = {}

            def emit_ln_A(lnps, t):
                sl = ts(t, 512)
                g = lnps.tile([128, 1024], dt, tag="ln")
                xb = gx_pool.tile([128, 512], dtb, tag="xb")
                xsq = gx_pool.tile([128, 512], dtb, tag="xsq")
                nc.gpsimd.tensor_copy(xb[:], x_sb[:, sl])
                nc.gpsimd.tensor_tensor(xsq[:], x_sb[:, sl], x_sb[:, sl], Op.mult)
                nc.tensor.matmul(g[0:1, 0:512], onesC[:, 0:1], xb[:])
                nc.tensor.matmul(g[0:1, 512:1024], onesC[:, 0:1], xsq[:])
                ln_state[t] = g

            def emit_ln_B(t):
                g = ln_state[t]
                # mean to SBUF first: DVE ops may read at most one PSUM
                # operand (single PSUM read port).
                mcp = sm_pool.tile([1, 512], dt, tag="mcp")
                msq = sm_pool.tile([1, 512], dt, tag="msq")
                var = sm_pool.tile([1, 512], dt, tag="var")
                lnv = sm_pool.tile([1, 512], dt, tag="lnv")
                ru = sm_pool.tile([1, 1024], dt, tag="ru")
                nc.vector.tensor_copy(mcp[:], g[0:1, 0:512])
                nc.vector.tensor_tensor(msq[:], mcp[:], mcp[:], Op.mult)
                nc.vector.scalar_tensor_tensor(
                    var[:], g[0:1, 512:1024], 1.0, msq[:], Op.mult, Op.subtract
                )
                nc.scalar.activation(lnv[:], var[:], F.Ln, bias=epsc[0:1, 0:1])
                nc.scalar.activation(ru[0:1, 0:512], lnv[:], F.Exp, scale=-0.5)
                # u = mean * rstd
                nc.vector.tensor_tensor(
                    ru[0:1, 512:1024], mcp[:], ru[0:1, 0:512], Op.mult
                )
                ln_state[t] = ru

            def emit_ln_C(lnps, t):
                sl = ts(t, 512)
                ru = ln_state.pop(t)
                bc = lnps.tile([128, 1024], dt, tag="ln")
                nc.tensor.matmul(bc[:, 0:512], ones1[0:1, :], ru[0:1, 0:512])
                nc.tensor.matmul(bc[:, 512:1024], ones1[0:1, :], ru[0:1, 512:1024])
                tmp = gx_pool.tile([128, 512], dt, tag="xtmp")
                nc.vector.tensor_tensor(tmp[:], x_sb[:, sl], bc[:, 0:512], Op.mult)
                nc.vector.tensor_tensor(
                    xn[:, sl], tmp[:], bc[:, 512:1024], Op.subtract
                )
                qk = lnps.tile([128, 1024], dt, tag="ln")
                if t < NIT:
                    nc.tensor.matmul(qk[:, 0:512], wq[:], xn[:, sl])
                    nc.vector.tensor_scalar(
                        q_sb[:, sl], qk[:, 0:512], bq[:, 0:1], None, Op.add
                    )
                    nc.vector.tensor_scalar(
                        qp3[96:128, sl], qk[96:128, 0:512], bq[96:128, 0:1],
                        None, Op.add,
                    )
                nc.tensor.matmul(qk[:, 512:1024], wk[:], xn[:, sl])
                nc.vector.tensor_copy(k_sb[:, sl], qk[:, 512:1024])
                vp = lnps.tile([128, 1024], dt, tag="ln")
                for c in range(4):
                    jc = 4 * t + c
                    nc.tensor.matmul(
                        vp[:, 128 * c : 128 * c + 128], xn[:, ts(jc, 128)], wv[:]
                    )
                dst = v_sb[
                    :, HEADS * VW * 4 * t : HEADS * VW * 4 * (t + 1)
                ].rearrange("p (c h e) -> p c h e", h=HEADS, e=VW)[:, :, :, 0:DIM_HEAD]
                src = vp[:, 0:512].rearrange(
                    "p (c h e) -> p c h e", h=HEADS, e=DIM_HEAD
                )
                nc.vector.tensor_copy(dst, src)

            # ---------------- attention ----------------
            # blocks stream per jc in head order (0, 2, 1, 3); duo tiles hold
            # `nb` 512-col blocks, exp'd in one ACT op, then consumed by
            # per-block av matmuls into the pair banks.
            H_ROWS = {0: ("A", 0), 2: ("A", 64), 1: ("B", 0), 3: ("B", 64)}

            def run_it(it, duop, duo_cols, lnps, interleave):
                isl = ts(it, TI)
                nb = duo_cols // 512
                pairA = pair_pool.tile([128, TI], dt, tag="pair")
                pairB = pair_pool.tile([128, TI], dt, tag="pair")
                pairs = {"A": pairA, "B": pairB}

                blocks = [(jc, h) for jc in range(NJC) for h in (0, 2, 1, 3)]
                groups = [blocks[i : i + nb] for i in range(0, len(blocks), nb)]

                def emit_sims(g):
                    dtile = duop.tile([128, duo_cols], dt, tag="duo")
                    for i, (jc, h) in enumerate(g):
                        if h == 3:
                            lhsT = k_sb[64:128, ts(jc, JCHUNK)]
                            rhs = qp3[64:128, isl]
                        else:
                            lhsT = k_sb[32 * h : 32 * h + 32, ts(jc, JCHUNK)]
                            rhs = q_sb[32 * h : 32 * h + 32, isl]
                        nc.tensor.matmul(
                            dtile[:, 512 * i : 512 * i + 512], lhsT, rhs
                        )
                    return dtile

                def emit_exp(dtile, g):
                    e = expo_pool.tile([128, 1536], dtb, tag="expo")
                    n = 512 * len(g)
                    nc.scalar.activation(e[:, 0:n], dtile[:, 0:n], F.Exp)
                    return e

                def emit_avs(e, g):
                    for i, (jc, h) in enumerate(g):
                        pid, rb = H_ROWS[h]
                        vcol = (HEADS * jc + h) * VW
                        nc.tensor.matmul(
                            pairs[pid][rb : rb + VW, :],
                            v_sb[:, vcol : vcol + VW],
                            e[:, 512 * i : 512 * i + 512],
                            start=jc == 0,
                            stop=jc == NJC - 1,
                            skip_group_check=True,
                        )

                prev = None
                for g in groups:
                    if interleave:
                        jc0, h0 = g[0]
                        if h0 == 0 and jc0 % 4 and jc0 // 4 + 1 < 8:
                            t = jc0 // 4 + 1
                            if jc0 % 4 == 1:
                                emit_ln_A(lnps, t)
                            elif jc0 % 4 == 2:
                                emit_ln_B(t)
                            else:
                                emit_ln_C(lnps, t)
                    d = emit_sims(g)
                    e = emit_exp(d, g)
                    if prev is not None:
                        emit_avs(*prev)
                    prev = (e, g)
                emit_avs(*prev)

                # tail: normalize + project. No ACT ops.
                if lnps is not None:
                    bcy = lnps.tile([128, 1024], dt, tag="ln")
                else:
                    bcy = duop.tile([128, duo_cols], dt, tag="duo")
                bc = bcy[0:97, 0:512]
                yp = bcy[0:128, 512:1024]
                for pi, (pair, cat, rec, wo) in enumerate(
                    ((pairA, catA, recA, woa), (pairB, catB, recB, wob))
                ):
                    bcs = rec_pool.tile([97, 512], dt, tag="bcs")
                    nc.vector.reciprocal(rec[0:1, :], pair[32:33, :])
                    nc.vector.reciprocal(rec[32:33, :], pair[96:97, :])
                    nc.tensor.matmul(bc, ones2[0:33, :], rec[0:33, :])
                    # bc to SBUF: cat = pair * bc would be two PSUM reads
                    nc.vector.tensor_copy(bcs[0:33, :], bc[0:33, :])
                    nc.vector.tensor_copy(bcs[64:97, :], bc[64:97, :])
                    nc.vector.tensor_tensor(
                        cat[0:33, :], pair[0:33, :], bcs[0:33, :], Op.mult
                    )
                    nc.vector.tensor_tensor(
                        cat[64:97, :], pair[64:97, :], bcs[64:97, :], Op.mult
                    )
                    nc.tensor.matmul(
                        yp, wo[:, :], cat[0:97, :], start=pi == 0, stop=pi == 1
                    )
                ysb = ysb_pool.tile([128, TI], dt, tag="ysb")
                nc.vector.tensor_scalar(ysb[:], yp, bo[:, 0:1], None, Op.add)
                nc.sync.dma_start(y_d[:, isl], ysb[:])

            with (
                tc.tile_pool(name="lnps", bufs=1, space="PSUM") as lnps,
                tc.tile_pool(name="duo0", bufs=2, space="PSUM") as duop0,
            ):
                emit_ln_A(lnps, 0)
                emit_ln_B(0)
                emit_ln_C(lnps, 0)
                run_it(0, duop0, 1024, lnps, interleave=True)
            with tc.tile_pool(name="duo1", bufs=2, space="PSUM") as duop1:
                for it in range(1, NIT):
                    run_it(it, duop1, 1536, None, interleave=False)

    nc.compile()
    return nc


def _get_program():
    global _PROGRAM
    if _PROGRAM is None:
        _PROGRAM = _build_program()
    return _PROGRAM


def _prep_inputs(x, g, b, w_qkv, w_out, b_out):
    """Host-side sharding + weight folding. All tiny except x slicing."""
    f32 = np.float32
    x = np.asarray(x, f32).reshape(B, C, S)
    g_ = np.asarray(g, f32).reshape(C)
    b_ = np.asarray(b, f32).reshape(C)
    w_qkv = np.asarray(w_qkv, f32)
    w_out = np.asarray(w_out, f32)
    b_out = np.asarray(b_out, f32)

    import ml_dtypes

    bf16 = ml_dtypes.bfloat16
    scale = DIM_HEAD ** -0.5
    wg = w_qkv * g_[None, :]
    bias_qkv = w_qkv @ b_
    hid = HEADS * DIM_HEAD  # 128
    wq_t = np.ascontiguousarray((wg[0:hid] * scale).T).astype(bf16)
    wk_t = np.ascontiguousarray(wg[hid : 2 * hid].T).astype(bf16)
    wv_t = np.ascontiguousarray(wg[2 * hid : 3 * hid].T).astype(bf16)
    bias_q = np.ascontiguousarray((bias_qkv[0:hid] * scale).reshape(128, 1))
    # bias_k is dropped: it shifts all logits of a query equally, and
    # softmax is shift-invariant. bias_v folds exactly into the output
    # bias (attention rows sum to 1).
    bias_v = bias_qkv[2 * hid : 3 * hid]

    wo_t = w_out.T  # [hd, o]
    wo_a = np.zeros((97, 128), f32)
    wo_b = np.zeros((97, 128), f32)
    wo_a[0:32] = wo_t[0:32]     # head 0
    wo_a[64:96] = wo_t[64:96]   # head 2
    wo_b[0:32] = wo_t[32:64]    # head 1
    wo_b[64:96] = wo_t[96:128]  # head 3
    bias_o = np.ascontiguousarray((b_out + w_out @ bias_v).reshape(128, 1))

    shared = {
        "wq_t": wq_t,
        "wk_t": wk_t,
        "wv_t": wv_t,
        "wo_a": wo_a,
        "wo_b": wo_b,
        "bias_q": bias_q,
        "bias_o": bias_o,
    }
    in_maps = []
    for core in range(N_CORES):
        bb, half = core // 2, core % 2
        if half == 0:
            xc = x[bb]
        else:
            xc = np.concatenate([x[bb][:, HALF:], x[bb][:, :HALF]], axis=1)
        m = {"x": np.ascontiguousarray(xc)}
        m.update(shared)
        in_maps.append(m)
    return in_maps


def _run(inputs, trace=False):
    from concourse.bass_utils import run_bass_kernel_spmd

    nc = _get_program()
    in_maps = _prep_inputs(**inputs)
    res = run_bass_kernel_spmd(
        nc, in_maps, core_ids=list(range(N_CORES)), trace=trace
    )
    y = np.empty((B, C, S), np.float32)
    for core in range(N_CORES):
        bb, half = core // 2, core % 2
        yc = res.results[core]["y"]
        if half == 0:
            y[bb][:, :HALF] = yc
        else:
            y[bb][:, HALF:] = yc
    return y.reshape(B, C, H, W), res


def kernel(x, g, b, w_qkv, w_out, b_out):
    out, _ = _run(
        {"x": x, "g": g, "b": b, "w_qkv": w_qkv, "w_out": w_out, "b_out": b_out}
    )
    return out
